# revision 30
# baseline (speedup 1.0000x reference)
"""Trainium2 Bass kernel for nn_LoraAttention (v4, scalar-bound design).

Math (reference): qkv = x@W_qkv.T; lora full proj ql/vl = split(x@W_lora.T +
b_lora) (K-part discarded); low-rank dq = (x@A_q.T)@B_q.T/8 (same for v);
softmax attention over H=16 heads, D=64; out = attn_cat@W_out.T + b_out.

Host-side algebra folds every LoRA term into the projection weights:
  Wq_eff = W_qkv[q] + W_lora[q] + (B_q@A_q)/8      (q bias b_lora[q] kept)
  Wk_eff = W_qkv[k]                                 (no bias)
  Wv_eff = W_qkv[v] + W_lora[v] + (B_v@A_v)/8
  b_eff  = b_out + W_out @ b_lora[v]   (v bias commutes through softmax)

Sharding: 8 cores = 4 batches x 2 head-groups (8 heads each). Each core
projects QKV for its heads, does attention, and computes a partial output
projection over its 512 concat dims; host sums the two partials per batch.

Device design (driven by trace analysis; ScalarE exp is the ~285us floor):
  - S^T per (pair,nq,mq): 2 row-packed matmuls (tile_position (0,0)/(64,0),
    K=64) -> 1 slot. exp on ScalarE [128,1024] fp16 out, with a -6 bias
    (softmax shift invariance) to keep exp in fp16 range (max logit ~13.6).
  - PV: 2 col-packed matmuls (tile_position (0,0)/(0,64), M=64) -> 1 slot;
    output atAB[128,512] is both heads' dims on partitions = exactly the
    output-projection layout.
  - softmax denominators: fp16 running sum of pe tiles on VectorE, reduced
    over partitions by two K=128->M=2 selector matmuls, reciprocal on DVE,
    broadcast back by one K=2 matmul, final scale on DVE.
  - PV emission lags S/exp by LAG=12 iterations (pe ring 16) and atAB is
    freed by a fast PSUM->SBUF cast, so ScalarE never waits on the
    normalize chain.
  - K/Q/V projections and the output projection run as micro-thunks (1-2
    matmuls each) popped from a queue after each iteration's S/exp, so no
    8-matmul burst ever delays the S that feeds ScalarE.
  - Inputs arrive in 10 large 3D-AP DMAs (SyncE dispatch costs ~0.6us per
    dma_start; v3 spent ~25us just issuing 46 input DMAs). Outputs fp16.
"""

import numpy as np

import concourse.bacc as bacc
import concourse.tile as tile
from concourse import mybir
from concourse.bass_utils import run_bass_kernel_spmd




B, N, C = 4, 2048, 1024
H, D = 16, 64
LORA_SCALE = 1.0 / 8.0
ATTN_SCALE = float(D) ** -0.5  # 0.125

f32 = mybir.dt.float32
f16 = mybir.dt.float16
F16 = np.float16

NQ = 4            # query chunks of 512
MQ = 16           # key chunks of 128
KC = 8            # contraction chunks of 128 over C
PAIRS = 4         # head pairs per core (8 local heads)
NITER = PAIRS * NQ * MQ   # 256
LAG = 14          # PV emission lag behind S/exp (iterations)
PERING = 20       # pe ring depth (> LAG + chain slack)

_cache: dict = {}


def _build_program():
    nc = bacc.Bacc("TRN2", target_bir_lowering=False, debug=False, num_devices=8)

    xT_d = nc.dram_tensor("xT", [C, N], f16, kind="ExternalInput").ap()
    wk_d = nc.dram_tensor("wk", [C, 512], f16, kind="ExternalInput").ap()
    wq_d = nc.dram_tensor("wq", [C, 512], f16, kind="ExternalInput").ap()
    wv_d = nc.dram_tensor("wv", [C, 512], f16, kind="ExternalInput").ap()
    wo_d = nc.dram_tensor("wo", [512, C], f16, kind="ExternalInput").ap()
    bq_d = nc.dram_tensor("bq", [128, 4], f32, kind="ExternalInput").ap()
    sel_d = nc.dram_tensor("sel", [2, 128], f16, kind="ExternalInput").ap()
    outT_d = nc.dram_tensor("outT", [C, N], f16, kind="ExternalOutput").ap()

    EXP = mybir.ActivationFunctionType.Exp

    with tile.TileContext(nc) as tc:
        with (
            tc.tile_pool(name="win", bufs=1) as win,        # weights + x + consts
            tc.tile_pool(name="kqp", bufs=1) as kqp,        # K/Q fp16 per pair
            tc.tile_pool(name="vp", bufs=1) as vp,          # V fp16 per key chunk
            tc.tile_pool(name="pex", bufs=PERING) as pex,   # exp outputs
            tc.tile_pool(name="esp", bufs=2) as esp,        # exp running sums
            tc.tile_pool(name="acp", bufs=1) as acp,        # normalized attn
            tc.tile_pool(name="scr", bufs=2) as scr,        # norm-chain scratch
            tc.tile_pool(name="osb", bufs=2) as osb,        # out eviction
            tc.tile_pool(name="spp", bufs=2, space="PSUM") as spp,   # S^T (4 banks)
            tc.tile_pool(name="app", bufs=1, space="PSUM") as app,   # PV accum (1)
            tc.tile_pool(name="ppp", bufs=2, space="PSUM") as ppp,   # proj/out (2)
            tc.tile_pool(name="aux", bufs=1, space="PSUM") as aux,   # den/rb (1)
        ):
            # ---------------- constants ----------------
            bqt = win.tile([128, 4], f32, tag="bq", name="bqt")
            selA = win.tile([128, 2], f16, tag="selA", name="selA")
            nc.vector.memset(selA[:, 0:1], 1.0)
            nc.vector.memset(selA[:, 1:2], 0.0)
            selB = win.tile([128, 2], f16, tag="selB", name="selB")
            nc.vector.memset(selB[:, 0:1], 0.0)
            nc.vector.memset(selB[:, 1:2], 1.0)
            sel128 = win.tile([2, 128], f16, tag="sel128", name="sel128")
            ebias = win.tile([128, 1], f32, tag="ebias", name="ebias")
            nc.vector.memset(ebias[:], -6.0)

            # ------- input DMAs: issued in parallel from SyncE + ScalarE ------
            # (each dma_start costs ~0.6-3us of issuing-engine time; the v5
            # single-engine stream pushed the first exp out to ~38us)
            xt = win.tile([128, KC, N], f16, tag="xt", name="xt")
            wkt = win.tile([128, KC, 512], f16, tag="wk", name="wkt")
            wqt = win.tile([128, KC, 512], f16, tag="wq", name="wqt")
            wvt = win.tile([128, KC, 512], f16, tag="wv", name="wvt")
            wot = win.tile([128, 4, 1024], f16, tag="wo", name="wot")
            xr = xT_d.rearrange("(kc p) n -> p kc n", kc=KC)
            wkr = wk_d.rearrange("(kc p) d -> p kc d", kc=KC)
            wqr = wq_d.rearrange("(kc p) d -> p kc d", kc=KC)
            wvr = wv_d.rearrange("(kc p) d -> p kc d", kc=KC)
            wor = wo_d.rearrange("(dc p) c -> p dc c", dc=4)
            nc.sync.dma_start(xt[:, :, 0:512], xr[:, :, 0:512])
            nc.sync.dma_start(wkt[:], wkr[:])
            nc.sync.dma_start(wqt[:], wqr[:])
            nc.sync.dma_start(bqt[:], bq_d[:])
            nc.sync.dma_start(sel128[:], sel_d[:])
            nc.sync.dma_start(xt[:, :, 512:1024], xr[:, :, 512:1024])
            nc.sync.dma_start(xt[:, :, 1024:2048], xr[:, :, 1024:2048])
            nc.sync.dma_start(wvt[:], wvr[:])
            nc.sync.dma_start(wot[:], wor[:])

            # ---------------- state ----------------
            kt, qt = {}, {}
            vts = [None] * MQ
            vready = [False] * MQ
            acat = [[None] * PAIRS for _ in range(NQ)]
            esums, atab, pe_ring = {}, {}, {}
            gq = []   # micro-thunk queue for background tensor work

            def ktile(t):
                if t not in kt:
                    kt[t] = kqp.tile([128, N], f16, tag=f"k{t}", name=f"kt{t}")
                    qt[t] = kqp.tile([128, N], f16, tag=f"q{t}", name=f"qt{t}")
                return kt[t], qt[t]

            def kq_group(t, j):
                """Returns micro-thunks: 4x(2 matmuls) + eviction."""
                kind, g = divmod(j, NQ)   # kind 0: K tokens g, 1: Q tokens g
                ktt, qtt = ktile(t)
                w = wkt if kind == 0 else wqt
                ps_box = []

                def mk(kc2):
                    def th():
                        if kc2 == 0:
                            ps_box.append(ppp.tile([128, 512], f32, tag="pp", name="ps"))
                        ps = ps_box[0]
                        for kc in (kc2, kc2 + 1):
                            nc.tensor.matmul(
                                ps[:], w[:, kc, t * 128:(t + 1) * 128],
                                xt[:, kc, g * 512:(g + 1) * 512],
                                start=(kc == 0), stop=(kc == KC - 1),
                            )
                    return th

                def evict():
                    ps = ps_box[0]
                    if kind == 0:
                        nc.vector.tensor_copy(ktt[:, g * 512:(g + 1) * 512], ps[:])
                    else:
                        nc.vector.tensor_scalar_add(
                            qtt[:, g * 512:(g + 1) * 512], ps[:], bqt[:, t:t + 1]
                        )
                return [mk(0), mk(2), mk(4), mk(6), evict]

            def v_group(m):
                vt = vp.tile([128, 512], f16, tag=f"v{m}", name=f"vt{m}")
                vts[m] = vt
                ps_box = []

                def mk(kc2):
                    def th():
                        if kc2 == 0:
                            ps_box.append(ppp.tile([128, 512], f32, tag="pp", name="ps"))
                        ps = ps_box[0]
                        for kc in (kc2, kc2 + 1):
                            nc.tensor.matmul(
                                ps[:], xt[:, kc, m * 128:(m + 1) * 128], wvt[:, kc, :],
                                start=(kc == 0), stop=(kc == KC - 1),
                            )
                    return th

                def evict():
                    nc.vector.tensor_copy(vt[:], ps_box[0][:])
                    vready[m] = True
                return [mk(0), mk(2), mk(4), mk(6), evict]

            def outproj_unit(nq, cc):
                ps_box = []

                def mk(d2):
                    def th():
                        if d2 == 0:
                            ps_box.append(ppp.tile([128, 512], f32, tag="pp", name="pso"))
                        ps = ps_box[0]
                        for dc in (d2, d2 + 1):
                            nc.tensor.matmul(
                                ps[:], wot[:, dc, cc * 128:(cc + 1) * 128],
                                acat[nq][dc][:],
                                start=(dc == 0), stop=(dc == 3),
                            )
                    return th

                def evict():
                    ob = osb.tile([128, 512], f16, tag="ob", name="ob")
                    nc.vector.tensor_copy(ob[:], ps_box[0][:])
                    nc.sync.dma_start(
                        outT_d[cc * 128:(cc + 1) * 128, nq * 512:(nq + 1) * 512], ob[:]
                    )
                return [mk(0), mk(2), evict]

            # ---------------- per-iteration pieces ----------------
            def emit_S_ACT(i):
                t, nq, m = i // 64, (i // 16) % 4, i % 16
                ktt, qtt = kt[t], qt[t]
                sp = spp.tile([128, 1024], f32, tag="sp", name="sp")
                # 4 disjoint (row,col) PE tiles -> all concurrent, 64-col LDWs
                q0, q1 = nq * 512, (nq + 1) * 512
                nc.tensor.matmul(
                    sp[0:64, 0:512], ktt[0:64, m * 128:m * 128 + 64],
                    qtt[0:64, q0:q1],
                    start=True, stop=True, tile_position=(0, 0),
                )
                nc.tensor.matmul(
                    sp[64:128, 0:512], ktt[0:64, m * 128 + 64:(m + 1) * 128],
                    qtt[0:64, q0:q1],
                    start=True, stop=True, tile_position=(0, 64),
                )
                nc.tensor.matmul(
                    sp[0:64, 512:1024], ktt[64:128, m * 128:m * 128 + 64],
                    qtt[64:128, q0:q1],
                    start=True, stop=True, tile_position=(64, 0),
                )
                nc.tensor.matmul(
                    sp[64:128, 512:1024], ktt[64:128, m * 128 + 64:(m + 1) * 128],
                    qtt[64:128, q0:q1],
                    start=True, stop=True, tile_position=(64, 64),
                )
                pe = pex.tile([128, 1024], f16, tag="pe", name="pe")
                # softmax shift invariance: exp(s/8 - 6) keeps fp16 in range
                nc.scalar.activation(pe[:], sp[:], EXP, bias=ebias[:, 0:1],
                                     scale=ATTN_SCALE)
                pe_ring[i] = pe
                u = i // 16
                if m == 0:
                    es = esp.tile([128, 1024], f16, tag="es", name="es")
                    esums[u] = es
                    nc.vector.tensor_copy(es[:], pe[:])
                else:
                    es = esums[u]
                    nc.vector.tensor_add(es[:], es[:], pe[:])

            def emit_PV(i):
                t, nq, m = i // 64, (i // 16) % 4, i % 16
                u = i // 16
                if m == 0:
                    atab[u] = app.tile([128, 512], f32, tag="at", name="atab")
                at = atab[u]
                vt = vts[m]
                pe = pe_ring.pop(i)
                nc.tensor.matmul(
                    at[0:64, :], vt[:, t * 128:t * 128 + 64], pe[:, 0:512],
                    start=(m == 0), stop=(m == MQ - 1), tile_position=(0, 0),
                )
                nc.tensor.matmul(
                    at[64:128, :], vt[:, t * 128 + 64:t * 128 + 128], pe[:, 512:1024],
                    start=(m == 0), stop=(m == MQ - 1), tile_position=(0, 64),
                )

            def emit_norm_a(u):
                """araw cast (frees atAB), den matmuls, reciprocal."""
                at = atab.pop(u)
                es = esums.pop(u)
                araw = scr.tile([128, 512], f16, tag="araw", name="araw")
                nc.vector.tensor_copy(araw[:], at[:])   # frees atAB fast
                den = aux.tile([2, 512], f32, tag="aux", name="den")
                nc.tensor.matmul(den[:], selA[:, :], es[:, 0:512],
                                 start=True, stop=False)
                nc.tensor.matmul(den[:], selB[:, :], es[:, 512:1024],
                                 start=False, stop=True)
                rr = scr.tile([2, 512], f32, tag="rr", name="rr")
                nc.vector.reciprocal_approx_fast(rr[:], den[:])
                rrh = scr.tile([2, 512], f16, tag="rrh", name="rrh")
                nc.vector.tensor_copy(rrh[:], rr[:])
                return araw, rrh

            def emit_norm_b(u, araw, rrh):
                """rb broadcast matmul (deferred so the DVE recip is done by
                the time it sits in the tensor queue), scale, publish acat."""
                t, nq = u // NQ, u % NQ
                rb = aux.tile([128, 512], f32, tag="aux", name="rb")
                nc.tensor.matmul(rb[:], sel128[:, :], rrh[:], start=True, stop=True)
                ac = acp.tile([128, 512], f16, tag=f"ac{u}", name="ac")
                nc.vector.tensor_mul(ac[:], araw[:], rb[:])
                acat[nq][t] = ac
                if t == PAIRS - 1:
                    for cc in range(8):
                        gq.extend(outproj_unit(nq, cc))

            # ---------------- prefetch schedule (enqueue iters) ----------------
            sched = {}

            def add(i, mk, *a):
                sched.setdefault(i, []).append((mk, a))

            add(1, kq_group, 0, 1)
            add(2, kq_group, 0, 2)
            add(3, kq_group, 0, 3)
            add(4, kq_group, 0, 5)
            vslots = [5, 6, 7, 8, 9, 10, 11, 13, 14, 15, 16, 17, 18, 19, 21, 22]
            for s, m in zip(vslots, range(0, MQ)):
                add(s, v_group, m)
            add(12, kq_group, 0, 6)
            add(20, kq_group, 0, 7)
            for t in range(1, PAIRS):
                for j in range(8):
                    add((t - 1) * 64 + 24 + 4 * j, kq_group, t, j)

            def gpop(i, n):
                for _ in range(n):
                    if not gq:
                        return
                    gq.pop(0)()

            # ---------------- main pipeline ----------------
            for th in kq_group(0, 0) + kq_group(0, 4):
                th()

            def lag_for(j):
                u = j // MQ
                lag = LAG - max(0, u - 9)   # taper to shrink the tail
                if j % MQ == 0:
                    lag += 2                # slack for araw-cast to free atAB
                return max(lag, 6)

            pend = []
            deferred = {}
            for i in range(NITER):
                for mk, a in sched.get(i, ()):
                    gq.extend(mk(*a))
                emit_S_ACT(i)
                if i in deferred:
                    emit_norm_b(*deferred.pop(i))
                while pend and pend[0] <= i - lag_for(pend[0]) and vready[pend[0] % 16]:
                    j = pend.pop(0)
                    emit_PV(j)
                    if j % MQ == MQ - 1:
                        u = j // MQ
                        araw, rr = emit_norm_a(u)
                        deferred[i + 2] = (u, araw, rr)
                pend.append(i)
                gpop(i, 5 if i < 40 else (8 if i > 240 else 3))
            while pend:
                j = pend.pop(0)
                while not vready[j % 16] and gq:
                    gpop(-1, 5)
                emit_PV(j)
                if j % MQ == MQ - 1:
                    u = j // MQ
                    araw, rr = emit_norm_a(u)
                    emit_norm_b(u, araw, rr)
            for i in sorted(deferred):
                emit_norm_b(*deferred.pop(i))
            while gq:
                gpop(-1, 8)

    nc.compile()
    return nc


def _get_program():
    if "nc" not in _cache:
        _cache["nc"] = _build_program()
    return _cache["nc"]


def _prep_in_maps(x, W_qkv, W_lora, b_lora, A_q, B_q, A_v, B_v, W_out):
    HD = H * D  # 1024
    Wq = W_qkv[0:HD] + W_lora[0:HD] + LORA_SCALE * (B_q @ A_q)
    Wk = W_qkv[HD:2 * HD]
    Wv = W_qkv[2 * HD:3 * HD] + W_lora[2 * HD:3 * HD] + LORA_SCALE * (B_v @ A_v)
    bq = b_lora[0:HD]

    xT = [np.ascontiguousarray(x[b].T).astype(F16) for b in range(B)]
    sel128 = np.zeros((2, 128), F16)
    sel128[0, 0:64] = 1.0
    sel128[1, 64:128] = 1.0
    in_maps = []
    for c in range(8):
        b, hg = divmod(c, 2)
        sel = slice(hg * 512, (hg + 1) * 512)
        in_maps.append({
            "xT": xT[b],
            "wk": np.ascontiguousarray(Wk[sel].T).astype(F16),
            "wq": np.ascontiguousarray(Wq[sel].T).astype(F16),
            "wv": np.ascontiguousarray(Wv[sel].T).astype(F16),
            "wo": np.ascontiguousarray(W_out[:, sel].T).astype(F16),
            "bq": np.ascontiguousarray(bq[sel].reshape(4, 128).T).astype(np.float32),
            "sel": sel128,
        })
    return in_maps


def kernel(x, W_qkv, W_lora, b_lora, A_q, B_q, A_v, B_v, W_out, b_out):
    x = np.asarray(x, np.float32)
    W_qkv = np.asarray(W_qkv, np.float32)
    W_lora = np.asarray(W_lora, np.float32)
    b_lora = np.asarray(b_lora, np.float32)
    A_q = np.asarray(A_q, np.float32)
    B_q = np.asarray(B_q, np.float32)
    A_v = np.asarray(A_v, np.float32)
    B_v = np.asarray(B_v, np.float32)
    W_out = np.asarray(W_out, np.float32)
    b_out = np.asarray(b_out, np.float32)

    in_maps = _prep_in_maps(x, W_qkv, W_lora, b_lora, A_q, B_q, A_v, B_v, W_out)
    b_eff = b_out + W_out @ b_lora[2 * H * D:3 * H * D]

    nc = _get_program()
    res = run_bass_kernel_spmd(nc, in_maps, list(range(8)))

    out = np.empty((B, N, C), np.float32)
    for b in range(B):
        acc = res.results[2 * b]["outT"].astype(np.float32)
        acc += res.results[2 * b + 1]["outT"].astype(np.float32)
        acc += b_eff[:, None]
        out[b] = acc.T
    return out


# revision 32
# speedup vs baseline: 1.0839x; 1.0839x over previous
"""Trainium2 Bass kernel for nn_LoraAttention (v4, scalar-bound design).

Math (reference): qkv = x@W_qkv.T; lora full proj ql/vl = split(x@W_lora.T +
b_lora) (K-part discarded); low-rank dq = (x@A_q.T)@B_q.T/8 (same for v);
softmax attention over H=16 heads, D=64; out = attn_cat@W_out.T + b_out.

Host-side algebra folds every LoRA term into the projection weights:
  Wq_eff = W_qkv[q] + W_lora[q] + (B_q@A_q)/8      (q bias b_lora[q] kept)
  Wk_eff = W_qkv[k]                                 (no bias)
  Wv_eff = W_qkv[v] + W_lora[v] + (B_v@A_v)/8
  b_eff  = b_out + W_out @ b_lora[v]   (v bias commutes through softmax)

Sharding: 8 cores = 4 batches x 2 head-groups (8 heads each). Each core
projects QKV for its heads, does attention, and computes a partial output
projection over its 512 concat dims; host sums the two partials per batch.

Device design (driven by trace analysis; ScalarE exp is the ~285us floor):
  - S^T per (pair,nq,mq): 2 row-packed matmuls (tile_position (0,0)/(64,0),
    K=64) -> 1 slot. exp on ScalarE [128,1024] fp16 out, with a -6 bias
    (softmax shift invariance) to keep exp in fp16 range (max logit ~13.6).
  - PV: 2 col-packed matmuls (tile_position (0,0)/(0,64), M=64) -> 1 slot;
    output atAB[128,512] is both heads' dims on partitions = exactly the
    output-projection layout.
  - softmax denominators: fp16 running sum of pe tiles on VectorE, reduced
    over partitions by two K=128->M=2 selector matmuls, reciprocal on DVE,
    broadcast back by one K=2 matmul, final scale on DVE.
  - PV emission lags S/exp by LAG=12 iterations (pe ring 16) and atAB is
    freed by a fast PSUM->SBUF cast, so ScalarE never waits on the
    normalize chain.
  - K/Q/V projections and the output projection run as micro-thunks (1-2
    matmuls each) popped from a queue after each iteration's S/exp, so no
    8-matmul burst ever delays the S that feeds ScalarE.
  - Inputs arrive in 10 large 3D-AP DMAs (SyncE dispatch costs ~0.6us per
    dma_start; v3 spent ~25us just issuing 46 input DMAs). Outputs fp16.
"""

import numpy as np

import concourse.bacc as bacc
import concourse.tile as tile
from concourse import mybir
from concourse.bass_utils import run_bass_kernel_spmd




B, N, C = 4, 2048, 1024
H, D = 16, 64
LORA_SCALE = 1.0 / 8.0
ATTN_SCALE = float(D) ** -0.5  # 0.125

f32 = mybir.dt.float32
f16 = mybir.dt.float16
F16 = np.float16

NQ = 4            # query chunks of 512
MQ = 16           # key chunks of 128
KC = 8            # contraction chunks of 128 over C
PAIRS = 4         # head pairs per core (8 local heads)
NITER = PAIRS * NQ * MQ   # 256
LAG = 14          # PV emission lag behind S/exp (iterations)
PERING = 20       # pe ring depth (> LAG + chain slack)

_cache: dict = {}


def _build_program():
    nc = bacc.Bacc("TRN2", target_bir_lowering=False, debug=False, num_devices=8)

    xT_d = nc.dram_tensor("xT", [C, N], f16, kind="ExternalInput").ap()
    wk_d = nc.dram_tensor("wk", [C, 512], f16, kind="ExternalInput").ap()
    wq_d = nc.dram_tensor("wq", [C, 512], f16, kind="ExternalInput").ap()
    wv_d = nc.dram_tensor("wv", [C, 512], f16, kind="ExternalInput").ap()
    wo_d = nc.dram_tensor("wo", [512, C], f16, kind="ExternalInput").ap()
    bq_d = nc.dram_tensor("bq", [128, 4], f32, kind="ExternalInput").ap()
    sel_d = nc.dram_tensor("sel", [2, 128], f16, kind="ExternalInput").ap()
    outT_d = nc.dram_tensor("outT", [C, N], f16, kind="ExternalOutput").ap()

    EXP = mybir.ActivationFunctionType.Exp

    with tile.TileContext(nc) as tc:
        with (
            tc.tile_pool(name="win", bufs=1) as win,        # weights + x + consts
            tc.tile_pool(name="kqp", bufs=1) as kqp,        # K/Q fp16 per pair
            tc.tile_pool(name="vp", bufs=1) as vp,          # V fp16 per key chunk
            tc.tile_pool(name="pex", bufs=PERING) as pex,   # exp outputs
            tc.tile_pool(name="esp", bufs=2) as esp,        # exp running sums
            tc.tile_pool(name="acp", bufs=1) as acp,        # normalized attn
            tc.tile_pool(name="scr", bufs=2) as scr,        # norm-chain scratch
            tc.tile_pool(name="osb", bufs=2) as osb,        # out eviction
            tc.tile_pool(name="spp", bufs=2, space="PSUM") as spp,   # S^T (4 banks)
            tc.tile_pool(name="app", bufs=1, space="PSUM") as app,   # PV accum (1)
            tc.tile_pool(name="ppp", bufs=2, space="PSUM") as ppp,   # proj/out (2)
            tc.tile_pool(name="aux", bufs=1, space="PSUM") as aux,   # den/rb (1)
        ):
            # ---------------- constants ----------------
            bqt = win.tile([128, 4], f32, tag="bq", name="bqt")
            selA = win.tile([128, 2], f16, tag="selA", name="selA")
            nc.vector.memset(selA[:, 0:1], 1.0)
            nc.vector.memset(selA[:, 1:2], 0.0)
            selB = win.tile([128, 2], f16, tag="selB", name="selB")
            nc.vector.memset(selB[:, 0:1], 0.0)
            nc.vector.memset(selB[:, 1:2], 1.0)
            sel128 = win.tile([2, 128], f16, tag="sel128", name="sel128")
            ebias = win.tile([128, 1], f32, tag="ebias", name="ebias")
            nc.vector.memset(ebias[:], -6.0)

            # ------- input DMAs: issued in parallel from SyncE + ScalarE ------
            # (each dma_start costs ~0.6-3us of issuing-engine time; the v5
            # single-engine stream pushed the first exp out to ~38us)
            xt = win.tile([128, KC, N], f16, tag="xt", name="xt")
            wkt = win.tile([128, KC, 512], f16, tag="wk", name="wkt")
            wqt = win.tile([128, KC, 512], f16, tag="wq", name="wqt")
            wvt = win.tile([128, KC, 512], f16, tag="wv", name="wvt")
            wot = win.tile([128, 4, 1024], f16, tag="wo", name="wot")
            xr = xT_d.rearrange("(kc p) n -> p kc n", kc=KC)
            wkr = wk_d.rearrange("(kc p) d -> p kc d", kc=KC)
            wqr = wq_d.rearrange("(kc p) d -> p kc d", kc=KC)
            wvr = wv_d.rearrange("(kc p) d -> p kc d", kc=KC)
            wor = wo_d.rearrange("(dc p) c -> p dc c", dc=4)
            nc.sync.dma_start(xt[:, :, 0:512], xr[:, :, 0:512])
            nc.sync.dma_start(wkt[:], wkr[:])
            nc.sync.dma_start(wqt[:], wqr[:])
            nc.sync.dma_start(bqt[:], bq_d[:])
            nc.sync.dma_start(sel128[:], sel_d[:])
            nc.sync.dma_start(xt[:, :, 512:1024], xr[:, :, 512:1024])
            nc.sync.dma_start(wvt[:, 0:4, :], wvr[:, 0:4, :])
            nc.sync.dma_start(xt[:, :, 1024:2048], xr[:, :, 1024:2048])
            nc.sync.dma_start(wvt[:, 4:8, :], wvr[:, 4:8, :])
            nc.sync.dma_start(wot[:], wor[:])

            # ---------------- state ----------------
            kt, qt = {}, {}
            vts = [None] * MQ
            vready = [False] * MQ
            acat = [[None] * PAIRS for _ in range(NQ)]
            esums, atab, pe_ring = {}, {}, {}
            gq = []   # micro-thunk queue for background tensor work

            def ktile(t):
                if t not in kt:
                    kt[t] = kqp.tile([128, N], f16, tag=f"k{t}", name=f"kt{t}")
                    qt[t] = kqp.tile([128, N], f16, tag=f"q{t}", name=f"qt{t}")
                return kt[t], qt[t]

            def kq_group(t, j):
                """Returns micro-thunks: 4x(2 matmuls) + eviction."""
                kind, g = divmod(j, NQ)   # kind 0: K tokens g, 1: Q tokens g
                ktt, qtt = ktile(t)
                w = wkt if kind == 0 else wqt
                ps_box = []

                def mk(kc2):
                    def th():
                        if kc2 == 0:
                            ps_box.append(ppp.tile([128, 512], f32, tag="pp", name="ps"))
                        ps = ps_box[0]
                        for kc in (kc2, kc2 + 1):
                            nc.tensor.matmul(
                                ps[:], w[:, kc, t * 128:(t + 1) * 128],
                                xt[:, kc, g * 512:(g + 1) * 512],
                                start=(kc == 0), stop=(kc == KC - 1),
                            )
                    return th

                def evict():
                    ps = ps_box[0]
                    if kind == 0:
                        nc.vector.tensor_copy(ktt[:, g * 512:(g + 1) * 512], ps[:])
                    else:
                        nc.vector.tensor_scalar_add(
                            qtt[:, g * 512:(g + 1) * 512], ps[:], bqt[:, t:t + 1]
                        )
                return [mk(0), mk(2), mk(4), mk(6), evict]

            def v_group(m):
                vt = vp.tile([128, 512], f16, tag=f"v{m}", name=f"vt{m}")
                vts[m] = vt
                ps_box = []

                def mk(kc2):
                    def th():
                        if kc2 == 0:
                            ps_box.append(ppp.tile([128, 512], f32, tag="pp", name="ps"))
                        ps = ps_box[0]
                        for kc in (kc2, kc2 + 1):
                            nc.tensor.matmul(
                                ps[:], xt[:, kc, m * 128:(m + 1) * 128], wvt[:, kc, :],
                                start=(kc == 0), stop=(kc == KC - 1),
                            )
                    return th

                def evict():
                    nc.vector.tensor_copy(vt[:], ps_box[0][:])
                    vready[m] = True
                return [mk(0), mk(2), mk(4), mk(6), evict]

            def outproj_unit(nq, cc):
                ps_box = []

                def mk(d2):
                    def th():
                        if d2 == 0:
                            ps_box.append(ppp.tile([128, 512], f32, tag="pp", name="pso"))
                        ps = ps_box[0]
                        for dc in (d2, d2 + 1):
                            nc.tensor.matmul(
                                ps[:], wot[:, dc, cc * 128:(cc + 1) * 128],
                                acat[nq][dc][:],
                                start=(dc == 0), stop=(dc == 3),
                            )
                    return th

                def evict():
                    ob = osb.tile([128, 512], f16, tag="ob", name="ob")
                    nc.vector.tensor_copy(ob[:], ps_box[0][:])
                    nc.sync.dma_start(
                        outT_d[cc * 128:(cc + 1) * 128, nq * 512:(nq + 1) * 512], ob[:]
                    )
                return [mk(0), mk(2), evict]

            # ---------------- per-iteration pieces ----------------
            def emit_S_ACT(i):
                t, nq, m = i // 64, (i // 16) % 4, i % 16
                ktt, qtt = kt[t], qt[t]
                sp = spp.tile([128, 1024], f32, tag="sp", name="sp")
                nc.tensor.matmul(
                    sp[:, 0:512], ktt[0:64, m * 128:(m + 1) * 128],
                    qtt[0:64, nq * 512:(nq + 1) * 512],
                    start=True, stop=True, tile_position=(0, 0),
                )
                nc.tensor.matmul(
                    sp[:, 512:1024], ktt[64:128, m * 128:(m + 1) * 128],
                    qtt[64:128, nq * 512:(nq + 1) * 512],
                    start=True, stop=True, tile_position=(64, 0),
                )
                pe = pex.tile([128, 1024], f16, tag="pe", name="pe")
                # softmax shift invariance: exp(s/8 - 6) keeps fp16 in range
                nc.scalar.activation(pe[:], sp[:], EXP, bias=ebias[:, 0:1],
                                     scale=ATTN_SCALE)
                pe_ring[i] = pe
                u = i // 16
                if m == 0:
                    es = esp.tile([128, 1024], f16, tag="es", name="es")
                    esums[u] = es
                    nc.vector.tensor_copy(es[:], pe[:])
                else:
                    es = esums[u]
                    nc.vector.tensor_add(es[:], es[:], pe[:])

            def emit_PV(i):
                t, nq, m = i // 64, (i // 16) % 4, i % 16
                u = i // 16
                if m == 0:
                    atab[u] = app.tile([128, 512], f32, tag="at", name="atab")
                at = atab[u]
                vt = vts[m]
                pe = pe_ring.pop(i)
                nc.tensor.matmul(
                    at[0:64, :], vt[:, t * 128:t * 128 + 64], pe[:, 0:512],
                    start=(m == 0), stop=(m == MQ - 1), tile_position=(0, 0),
                )
                nc.tensor.matmul(
                    at[64:128, :], vt[:, t * 128 + 64:t * 128 + 128], pe[:, 512:1024],
                    start=(m == 0), stop=(m == MQ - 1), tile_position=(0, 64),
                )

            def emit_norm_a(u):
                """araw cast (frees atAB), den matmuls, reciprocal."""
                at = atab.pop(u)
                es = esums.pop(u)
                araw = scr.tile([128, 512], f16, tag="araw", name="araw")
                nc.vector.tensor_copy(araw[:], at[:])   # frees atAB fast
                den = aux.tile([2, 512], f32, tag="aux", name="den")
                nc.tensor.matmul(den[:], selA[:, :], es[:, 0:512],
                                 start=True, stop=False)
                nc.tensor.matmul(den[:], selB[:, :], es[:, 512:1024],
                                 start=False, stop=True)
                rr = scr.tile([2, 512], f32, tag="rr", name="rr")
                nc.vector.reciprocal_approx_fast(rr[:], den[:])
                rrh = scr.tile([2, 512], f16, tag="rrh", name="rrh")
                nc.vector.tensor_copy(rrh[:], rr[:])
                return araw, rrh

            def emit_norm_b(u, araw, rrh):
                """rb broadcast matmul (deferred so the DVE recip is done by
                the time it sits in the tensor queue), scale, publish acat."""
                t, nq = u // NQ, u % NQ
                rb = aux.tile([128, 512], f32, tag="aux", name="rb")
                nc.tensor.matmul(rb[:], sel128[:, :], rrh[:], start=True, stop=True)
                ac = acp.tile([128, 512], f16, tag=f"ac{u}", name="ac")
                nc.vector.tensor_mul(ac[:], araw[:], rb[:])
                acat[nq][t] = ac
                if t == PAIRS - 1:
                    for cc in range(8):
                        gq.extend(outproj_unit(nq, cc))

            # ---------------- prefetch schedule (enqueue iters) ----------------
            sched = {}

            def add(i, mk, *a):
                sched.setdefault(i, []).append((mk, a))

            add(1, kq_group, 0, 1)
            add(2, kq_group, 0, 2)
            add(3, kq_group, 0, 3)
            add(4, kq_group, 0, 5)
            vslots = [5, 6, 7, 8, 9, 10, 11, 13, 14, 15, 16, 17, 18, 19, 21, 22]
            for s, m in zip(vslots, range(0, MQ)):
                add(s, v_group, m)
            add(12, kq_group, 0, 6)
            add(20, kq_group, 0, 7)
            for t in range(1, PAIRS):
                for j in range(8):
                    add((t - 1) * 64 + 24 + 4 * j, kq_group, t, j)

            def gpop(i, n):
                for _ in range(n):
                    if not gq:
                        return
                    gq.pop(0)()

            # ---------------- main pipeline ----------------
            for th in kq_group(0, 0) + kq_group(0, 4):
                th()

            def lag_for(j):
                u = j // MQ
                lag = LAG - max(0, u - 9)   # taper to shrink the tail
                if j % MQ == 0:
                    lag += 2                # slack for araw-cast to free atAB
                return max(lag, 6)

            pend = []
            deferred = {}
            for i in range(NITER):
                for mk, a in sched.get(i, ()):
                    gq.extend(mk(*a))
                emit_S_ACT(i)
                if i in deferred:
                    emit_norm_b(*deferred.pop(i))
                while pend and pend[0] <= i - lag_for(pend[0]) and vready[pend[0] % 16]:
                    j = pend.pop(0)
                    emit_PV(j)
                    if j % MQ == MQ - 1:
                        u = j // MQ
                        araw, rr = emit_norm_a(u)
                        deferred[i + 2] = (u, araw, rr)
                pend.append(i)
                gpop(i, 5 if i < 40 else (8 if i > 240 else 3))
            while pend:
                j = pend.pop(0)
                while not vready[j % 16] and gq:
                    gpop(-1, 5)
                emit_PV(j)
                if j % MQ == MQ - 1:
                    u = j // MQ
                    araw, rr = emit_norm_a(u)
                    emit_norm_b(u, araw, rr)
            for i in sorted(deferred):
                emit_norm_b(*deferred.pop(i))
            while gq:
                gpop(-1, 8)

    nc.compile()
    return nc


def _get_program():
    if "nc" not in _cache:
        _cache["nc"] = _build_program()
    return _cache["nc"]


def _prep_in_maps(x, W_qkv, W_lora, b_lora, A_q, B_q, A_v, B_v, W_out):
    HD = H * D  # 1024
    Wq = W_qkv[0:HD] + W_lora[0:HD] + LORA_SCALE * (B_q @ A_q)
    Wk = W_qkv[HD:2 * HD]
    Wv = W_qkv[2 * HD:3 * HD] + W_lora[2 * HD:3 * HD] + LORA_SCALE * (B_v @ A_v)
    bq = b_lora[0:HD]

    xT = [np.ascontiguousarray(x[b].T).astype(F16) for b in range(B)]
    sel128 = np.zeros((2, 128), F16)
    sel128[0, 0:64] = 1.0
    sel128[1, 64:128] = 1.0
    in_maps = []
    for c in range(8):
        b, hg = divmod(c, 2)
        sel = slice(hg * 512, (hg + 1) * 512)
        in_maps.append({
            "xT": xT[b],
            "wk": np.ascontiguousarray(Wk[sel].T).astype(F16),
            "wq": np.ascontiguousarray(Wq[sel].T).astype(F16),
            "wv": np.ascontiguousarray(Wv[sel].T).astype(F16),
            "wo": np.ascontiguousarray(W_out[:, sel].T).astype(F16),
            "bq": np.ascontiguousarray(bq[sel].reshape(4, 128).T).astype(np.float32),
            "sel": sel128,
        })
    return in_maps


def kernel(x, W_qkv, W_lora, b_lora, A_q, B_q, A_v, B_v, W_out, b_out):
    x = np.asarray(x, np.float32)
    W_qkv = np.asarray(W_qkv, np.float32)
    W_lora = np.asarray(W_lora, np.float32)
    b_lora = np.asarray(b_lora, np.float32)
    A_q = np.asarray(A_q, np.float32)
    B_q = np.asarray(B_q, np.float32)
    A_v = np.asarray(A_v, np.float32)
    B_v = np.asarray(B_v, np.float32)
    W_out = np.asarray(W_out, np.float32)
    b_out = np.asarray(b_out, np.float32)

    in_maps = _prep_in_maps(x, W_qkv, W_lora, b_lora, A_q, B_q, A_v, B_v, W_out)
    b_eff = b_out + W_out @ b_lora[2 * H * D:3 * H * D]

    nc = _get_program()
    res = run_bass_kernel_spmd(nc, in_maps, list(range(8)))

    out = np.empty((B, N, C), np.float32)
    for b in range(B):
        acc = res.results[2 * b]["outT"].astype(np.float32)
        acc += res.results[2 * b + 1]["outT"].astype(np.float32)
        acc += b_eff[:, None]
        out[b] = acc.T
    return out


# revision 38
# speedup vs baseline: 1.0993x; 1.0142x over previous
"""Trainium2 Bass kernel for nn_LoraAttention (440us baseline -> ~363us).

Math (reference): qkv = x@W_qkv.T; lora full proj ql/vl = split(x@W_lora.T +
b_lora) (K-part discarded); low-rank dq = (x@A_q.T)@B_q.T/8 (same for v);
softmax attention over H=16 heads, D=64; out = attn_cat@W_out.T + b_out.

Host-side algebra folds every LoRA term into the projection weights:
  Wq_eff = W_qkv[q] + W_lora[q] + (B_q@A_q)/8      (q bias b_lora[q] kept)
  Wk_eff = W_qkv[k]                                 (no bias)
  Wv_eff = W_qkv[v] + W_lora[v] + (B_v@A_v)/8
  b_eff  = b_out + W_out @ b_lora[v]   (v bias commutes through softmax)

Sharding: 8 cores = 4 batches x 2 head-groups (8 heads each). Each core
projects QKV for its heads, does attention, and computes a partial output
projection over its 512 concat dims; host sums the two partials per batch.

Device design (driven by trace analysis; ScalarE exp is the ~285us floor):
  - S^T per (pair,nq,mq): 2 row-packed matmuls (tile_position (0,0)/(64,0),
    K=64) -> 1 slot. exp on ScalarE [128,1024] fp16 out, with a -6 bias
    (softmax shift invariance) to keep exp in fp16 range (max logit ~13.6).
  - PV: 2 col-packed matmuls (tile_position (0,0)/(0,64), M=64) -> 1 slot;
    output atAB[128,512] is both heads' dims on partitions = exactly the
    output-projection layout.
  - softmax denominators: fp16 running sum of pe tiles on VectorE, reduced
    over partitions by two K=128->M=2 selector matmuls, reciprocal on DVE,
    broadcast back by one K=2 matmul, final scale on DVE.
  - PV emission lags S/exp by LAG=12 iterations (pe ring 16) and atAB is
    freed by a fast PSUM->SBUF cast, so ScalarE never waits on the
    normalize chain.
  - K/Q/V projections and the output projection run as micro-thunks (2
    matmuls each) popped from a queue after each iteration's S/exp, so no
    8-matmul burst ever delays the S that feeds ScalarE.
  - Inputs arrive in 9 large 3D-AP DMAs ordered x(0:512)/wk/wq/x(512:1024)/
    wv/x(rest)/wo (SyncE dispatch costs ~0.6-3us per dma_start and only
    starts ~9.7us in; an early version spent ~25us just issuing 46 input
    DMAs). Outputs fp16 (halves the tail drain).
Known dead ends (measured): fp8 projections (dot-product quantization error
does not average down -> ~5% logits), splitting S into 4 concurrent 64x64
tiles (+26us), walrus --enable-ldw-opt (rejects bass LDWEIGHTS), DMAs issued
from ScalarE (device crash).
"""

import numpy as np

import concourse.bacc as bacc
import concourse.tile as tile
from concourse import mybir
from concourse.bass_utils import run_bass_kernel_spmd




B, N, C = 4, 2048, 1024
H, D = 16, 64
LORA_SCALE = 1.0 / 8.0
ATTN_SCALE = float(D) ** -0.5  # 0.125

f32 = mybir.dt.float32
f16 = mybir.dt.float16
F16 = np.float16

NQ = 4            # query chunks of 512
MQ = 16           # key chunks of 128
KC = 8            # contraction chunks of 128 over C
PAIRS = 4         # head pairs per core (8 local heads)
NITER = PAIRS * NQ * MQ   # 256
LAG = 14          # PV emission lag behind S/exp (iterations)
PERING = 20       # pe ring depth (> LAG + chain slack)

_cache: dict = {}


def _build_program():
    nc = bacc.Bacc("TRN2", target_bir_lowering=False, debug=False, num_devices=8)

    xT_d = nc.dram_tensor("xT", [C, N], f16, kind="ExternalInput").ap()
    wk_d = nc.dram_tensor("wk", [C, 512], f16, kind="ExternalInput").ap()
    wq_d = nc.dram_tensor("wq", [C, 512], f16, kind="ExternalInput").ap()
    wv_d = nc.dram_tensor("wv", [C, 512], f16, kind="ExternalInput").ap()
    wo_d = nc.dram_tensor("wo", [512, C], f16, kind="ExternalInput").ap()
    bq_d = nc.dram_tensor("bq", [128, 4], f32, kind="ExternalInput").ap()
    sel_d = nc.dram_tensor("sel", [2, 128], f16, kind="ExternalInput").ap()
    outT_d = nc.dram_tensor("outT", [C, N], f16, kind="ExternalOutput").ap()

    EXP = mybir.ActivationFunctionType.Exp

    with tile.TileContext(nc) as tc:
        with (
            tc.tile_pool(name="win", bufs=1) as win,        # weights + x + consts
            tc.tile_pool(name="kqp", bufs=1) as kqp,        # K/Q fp16 per pair
            tc.tile_pool(name="vp", bufs=1) as vp,          # V fp16 per key chunk
            tc.tile_pool(name="pex", bufs=PERING) as pex,   # exp outputs
            tc.tile_pool(name="esp", bufs=2) as esp,        # exp running sums
            tc.tile_pool(name="acp", bufs=1) as acp,        # normalized attn
            tc.tile_pool(name="scr", bufs=2) as scr,        # norm-chain scratch
            tc.tile_pool(name="osb", bufs=2) as osb,        # out eviction
            tc.tile_pool(name="spp", bufs=2, space="PSUM") as spp,   # S^T (4 banks)
            tc.tile_pool(name="app", bufs=1, space="PSUM") as app,   # PV accum (1)
            tc.tile_pool(name="ppp", bufs=2, space="PSUM") as ppp,   # proj/out (2)
            tc.tile_pool(name="aux", bufs=1, space="PSUM") as aux,   # den/rb (1)
        ):
            # ---------------- constants ----------------
            bqt = win.tile([128, 4], f32, tag="bq", name="bqt")
            selA = win.tile([128, 2], f16, tag="selA", name="selA")
            nc.vector.memset(selA[:, 0:1], 1.0)
            nc.vector.memset(selA[:, 1:2], 0.0)
            selB = win.tile([128, 2], f16, tag="selB", name="selB")
            nc.vector.memset(selB[:, 0:1], 0.0)
            nc.vector.memset(selB[:, 1:2], 1.0)
            sel128 = win.tile([2, 128], f16, tag="sel128", name="sel128")
            ebias = win.tile([128, 1], f32, tag="ebias", name="ebias")
            nc.vector.memset(ebias[:], -6.0)

            # ------- input DMAs: issued in parallel from SyncE + ScalarE ------
            # (each dma_start costs ~0.6-3us of issuing-engine time; the v5
            # single-engine stream pushed the first exp out to ~38us)
            xt = win.tile([128, KC, N], f16, tag="xt", name="xt")
            wkt = win.tile([128, KC, 512], f16, tag="wk", name="wkt")
            wqt = win.tile([128, KC, 512], f16, tag="wq", name="wqt")
            wvt = win.tile([128, KC, 512], f16, tag="wv", name="wvt")
            wot = win.tile([128, 4, 1024], f16, tag="wo", name="wot")
            xr = xT_d.rearrange("(kc p) n -> p kc n", kc=KC)
            wkr = wk_d.rearrange("(kc p) d -> p kc d", kc=KC)
            wqr = wq_d.rearrange("(kc p) d -> p kc d", kc=KC)
            wvr = wv_d.rearrange("(kc p) d -> p kc d", kc=KC)
            wor = wo_d.rearrange("(dc p) c -> p dc c", dc=4)
            nc.sync.dma_start(xt[:, :, 0:512], xr[:, :, 0:512])
            nc.sync.dma_start(wkt[:], wkr[:])
            nc.sync.dma_start(wqt[:], wqr[:])
            nc.sync.dma_start(bqt[:], bq_d[:])
            nc.sync.dma_start(sel128[:], sel_d[:])
            nc.sync.dma_start(xt[:, :, 512:1024], xr[:, :, 512:1024])
            nc.sync.dma_start(wvt[:], wvr[:])
            nc.sync.dma_start(xt[:, :, 1024:2048], xr[:, :, 1024:2048])
            nc.sync.dma_start(wot[:], wor[:])

            # ---------------- state ----------------
            kt, qt = {}, {}
            vts = [None] * MQ
            vready = [False] * MQ
            acat = [[None] * PAIRS for _ in range(NQ)]
            esums, atab, pe_ring = {}, {}, {}
            gq = []   # micro-thunk queue for background tensor work

            def ktile(t):
                if t not in kt:
                    kt[t] = kqp.tile([128, N], f16, tag=f"k{t}", name=f"kt{t}")
                    qt[t] = kqp.tile([128, N], f16, tag=f"q{t}", name=f"qt{t}")
                return kt[t], qt[t]

            def kq_group(t, j):
                """Returns micro-thunks: 4x(2 matmuls) + eviction."""
                kind, g = divmod(j, NQ)   # kind 0: K tokens g, 1: Q tokens g
                ktt, qtt = ktile(t)
                w = wkt if kind == 0 else wqt
                ps_box = []

                def mk(kc2):
                    def th():
                        if kc2 == 0:
                            ps_box.append(ppp.tile([128, 512], f32, tag="pp", name="ps"))
                        ps = ps_box[0]
                        for kc in (kc2, kc2 + 1):
                            nc.tensor.matmul(
                                ps[:], w[:, kc, t * 128:(t + 1) * 128],
                                xt[:, kc, g * 512:(g + 1) * 512],
                                start=(kc == 0), stop=(kc == KC - 1),
                            )
                    return th

                def evict():
                    ps = ps_box[0]
                    if kind == 0:
                        nc.vector.tensor_copy(ktt[:, g * 512:(g + 1) * 512], ps[:])
                    else:
                        nc.vector.tensor_scalar_add(
                            qtt[:, g * 512:(g + 1) * 512], ps[:], bqt[:, t:t + 1]
                        )
                return [mk(0), mk(2), mk(4), mk(6), evict]

            def v_group(m):
                vt = vp.tile([128, 512], f16, tag=f"v{m}", name=f"vt{m}")
                vts[m] = vt
                ps_box = []

                def mk(kc2):
                    def th():
                        if kc2 == 0:
                            ps_box.append(ppp.tile([128, 512], f32, tag="pp", name="ps"))
                        ps = ps_box[0]
                        for kc in (kc2, kc2 + 1):
                            nc.tensor.matmul(
                                ps[:], xt[:, kc, m * 128:(m + 1) * 128], wvt[:, kc, :],
                                start=(kc == 0), stop=(kc == KC - 1),
                            )
                    return th

                def evict():
                    nc.vector.tensor_copy(vt[:], ps_box[0][:])
                    vready[m] = True
                return [mk(0), mk(2), mk(4), mk(6), evict]

            def outproj_unit(nq, cc):
                ps_box = []

                def mk(d2):
                    def th():
                        if d2 == 0:
                            ps_box.append(ppp.tile([128, 512], f32, tag="pp", name="pso"))
                        ps = ps_box[0]
                        for dc in (d2, d2 + 1):
                            nc.tensor.matmul(
                                ps[:], wot[:, dc, cc * 128:(cc + 1) * 128],
                                acat[nq][dc][:],
                                start=(dc == 0), stop=(dc == 3),
                            )
                    return th

                def evict():
                    ob = osb.tile([128, 512], f16, tag="ob", name="ob")
                    nc.vector.tensor_copy(ob[:], ps_box[0][:])
                    nc.sync.dma_start(
                        outT_d[cc * 128:(cc + 1) * 128, nq * 512:(nq + 1) * 512], ob[:]
                    )
                return [mk(0), mk(2), evict]

            # ---------------- per-iteration pieces ----------------
            def emit_S_ACT(i):
                t, nq, m = i // 64, (i // 16) % 4, i % 16
                ktt, qtt = kt[t], qt[t]
                sp = spp.tile([128, 1024], f32, tag="sp", name="sp")
                nc.tensor.matmul(
                    sp[:, 0:512], ktt[0:64, m * 128:(m + 1) * 128],
                    qtt[0:64, nq * 512:(nq + 1) * 512],
                    start=True, stop=True, tile_position=(0, 0),
                )
                nc.tensor.matmul(
                    sp[:, 512:1024], ktt[64:128, m * 128:(m + 1) * 128],
                    qtt[64:128, nq * 512:(nq + 1) * 512],
                    start=True, stop=True, tile_position=(64, 0),
                )
                pe = pex.tile([128, 1024], f16, tag="pe", name="pe")
                # softmax shift invariance: exp(s/8 - 6) keeps fp16 in range
                nc.scalar.activation(pe[:], sp[:], EXP, bias=ebias[:, 0:1],
                                     scale=ATTN_SCALE)
                pe_ring[i] = pe
                u = i // 16
                if m == 0:
                    es = esp.tile([128, 1024], f16, tag="es", name="es")
                    esums[u] = es
                    nc.vector.tensor_copy(es[:], pe[:])
                else:
                    es = esums[u]
                    nc.vector.tensor_add(es[:], es[:], pe[:])

            def emit_PV(i):
                t, nq, m = i // 64, (i // 16) % 4, i % 16
                u = i // 16
                if m == 0:
                    atab[u] = app.tile([128, 512], f32, tag="at", name="atab")
                at = atab[u]
                vt = vts[m]
                pe = pe_ring.pop(i)
                nc.tensor.matmul(
                    at[0:64, :], vt[:, t * 128:t * 128 + 64], pe[:, 0:512],
                    start=(m == 0), stop=(m == MQ - 1), tile_position=(0, 0),
                )
                nc.tensor.matmul(
                    at[64:128, :], vt[:, t * 128 + 64:t * 128 + 128], pe[:, 512:1024],
                    start=(m == 0), stop=(m == MQ - 1), tile_position=(0, 64),
                )

            def emit_norm_a(u):
                """araw cast (frees atAB), den matmuls, reciprocal."""
                at = atab.pop(u)
                es = esums.pop(u)
                araw = scr.tile([128, 512], f16, tag="araw", name="araw")
                nc.vector.tensor_copy(araw[:], at[:])   # frees atAB fast
                den = aux.tile([2, 512], f32, tag="aux", name="den")
                nc.tensor.matmul(den[:], selA[:, :], es[:, 0:512],
                                 start=True, stop=False)
                nc.tensor.matmul(den[:], selB[:, :], es[:, 512:1024],
                                 start=False, stop=True)
                rr = scr.tile([2, 512], f32, tag="rr", name="rr")
                nc.vector.reciprocal_approx_fast(rr[:], den[:])
                rrh = scr.tile([2, 512], f16, tag="rrh", name="rrh")
                nc.vector.tensor_copy(rrh[:], rr[:])
                return araw, rrh

            def emit_norm_b(u, araw, rrh):
                """rb broadcast matmul (deferred so the DVE recip is done by
                the time it sits in the tensor queue), scale, publish acat."""
                t, nq = u // NQ, u % NQ
                rb = aux.tile([128, 512], f32, tag="aux", name="rb")
                nc.tensor.matmul(rb[:], sel128[:, :], rrh[:], start=True, stop=True)
                ac = acp.tile([128, 512], f16, tag=f"ac{u}", name="ac")
                nc.vector.tensor_mul(ac[:], araw[:], rb[:])
                acat[nq][t] = ac
                if t == PAIRS - 1:
                    for cc in range(8):
                        gq.extend(outproj_unit(nq, cc))

            # ---------------- prefetch schedule (enqueue iters) ----------------
            sched = {}

            def add(i, mk, *a):
                sched.setdefault(i, []).append((mk, a))

            add(1, kq_group, 0, 1)
            vslots = [2, 3, 5, 6, 8, 9, 11, 12, 13, 14, 16, 17, 18, 19]
            for s, m in zip(vslots, range(2, MQ)):
                add(s, v_group, m)
            add(4, kq_group, 0, 2)
            add(7, kq_group, 0, 3)
            add(10, kq_group, 0, 5)
            add(15, kq_group, 0, 6)
            add(20, kq_group, 0, 7)
            for t in range(1, PAIRS):
                for j in range(8):
                    add((t - 1) * 64 + 22 + 4 * j, kq_group, t, j)

            def gpop(i, n):
                for _ in range(n):
                    if not gq:
                        return
                    gq.pop(0)()

            # ---------------- main pipeline ----------------
            for th in kq_group(0, 0) + kq_group(0, 4) + v_group(0) + v_group(1):
                th()

            def lag_for(j):
                u = j // MQ
                lag = LAG - max(0, u - 9)   # taper to shrink the tail
                if j % MQ == 0:
                    lag += 2                # slack for araw-cast to free atAB
                return max(lag, 6)

            pend = []
            deferred = {}
            for i in range(NITER):
                for mk, a in sched.get(i, ()):
                    gq.extend(mk(*a))
                emit_S_ACT(i)
                if i in deferred:
                    emit_norm_b(*deferred.pop(i))
                while pend and pend[0] <= i - lag_for(pend[0]) and vready[pend[0] % 16]:
                    j = pend.pop(0)
                    emit_PV(j)
                    if j % MQ == MQ - 1:
                        u = j // MQ
                        araw, rr = emit_norm_a(u)
                        deferred[i + 2] = (u, araw, rr)
                pend.append(i)
                gpop(i, 5 if i < 40 else 3)
            while pend:
                j = pend.pop(0)
                while not vready[j % 16] and gq:
                    gpop(-1, 5)
                emit_PV(j)
                if j % MQ == MQ - 1:
                    u = j // MQ
                    araw, rr = emit_norm_a(u)
                    emit_norm_b(u, araw, rr)
            for i in sorted(deferred):
                emit_norm_b(*deferred.pop(i))
            while gq:
                gpop(-1, 8)

    nc.compile()
    return nc


def _get_program():
    if "nc" not in _cache:
        _cache["nc"] = _build_program()
    return _cache["nc"]


def _prep_in_maps(x, W_qkv, W_lora, b_lora, A_q, B_q, A_v, B_v, W_out):
    HD = H * D  # 1024
    Wq = W_qkv[0:HD] + W_lora[0:HD] + LORA_SCALE * (B_q @ A_q)
    Wk = W_qkv[HD:2 * HD]
    Wv = W_qkv[2 * HD:3 * HD] + W_lora[2 * HD:3 * HD] + LORA_SCALE * (B_v @ A_v)
    bq = b_lora[0:HD]

    xT = [np.ascontiguousarray(x[b].T).astype(F16) for b in range(B)]
    sel128 = np.zeros((2, 128), F16)
    sel128[0, 0:64] = 1.0
    sel128[1, 64:128] = 1.0
    in_maps = []
    for c in range(8):
        b, hg = divmod(c, 2)
        sel = slice(hg * 512, (hg + 1) * 512)
        in_maps.append({
            "xT": xT[b],
            "wk": np.ascontiguousarray(Wk[sel].T).astype(F16),
            "wq": np.ascontiguousarray(Wq[sel].T).astype(F16),
            "wv": np.ascontiguousarray(Wv[sel].T).astype(F16),
            "wo": np.ascontiguousarray(W_out[:, sel].T).astype(F16),
            "bq": np.ascontiguousarray(bq[sel].reshape(4, 128).T).astype(np.float32),
            "sel": sel128,
        })
    return in_maps


def kernel(x, W_qkv, W_lora, b_lora, A_q, B_q, A_v, B_v, W_out, b_out):
    x = np.asarray(x, np.float32)
    W_qkv = np.asarray(W_qkv, np.float32)
    W_lora = np.asarray(W_lora, np.float32)
    b_lora = np.asarray(b_lora, np.float32)
    A_q = np.asarray(A_q, np.float32)
    B_q = np.asarray(B_q, np.float32)
    A_v = np.asarray(A_v, np.float32)
    B_v = np.asarray(B_v, np.float32)
    W_out = np.asarray(W_out, np.float32)
    b_out = np.asarray(b_out, np.float32)

    in_maps = _prep_in_maps(x, W_qkv, W_lora, b_lora, A_q, B_q, A_v, B_v, W_out)
    b_eff = b_out + W_out @ b_lora[2 * H * D:3 * H * D]

    nc = _get_program()
    res = run_bass_kernel_spmd(nc, in_maps, list(range(8)))

    out = np.empty((B, N, C), np.float32)
    for b in range(B):
        acc = res.results[2 * b]["outT"].astype(np.float32)
        acc += res.results[2 * b + 1]["outT"].astype(np.float32)
        acc += b_eff[:, None]
        out[b] = acc.T
    return out
